# revision 1
# baseline (speedup 1.0000x reference)
"""Multi-head self-attention on 8 Trainium2 NeuronCores.

Sharding: core i handles batch b = i // 4 and head-group g = i % 4
(4 of 16 heads).  Tensor-parallel over heads for the QKV/attention/output
projection, data-parallel over batch.  Each core produces a partial
output (its head-group's slice of the final projection); the all-reduce
over the 4 head-group partials per batch happens on the host after the
gather, together with adding the output bias exactly once.

Device layout notes:
  - Host pre-transposes x to xT and pre-packs every tensor into the
    exact [128, F] SBUF image the kernel DMAs, so the device never
    transposes anything.
  - QK matmuls are fp32r (full-rate single-pass fp32); the PV side runs
    in bf16: exp(scores) is produced in bf16 both by the ACT engine
    (exact spline exp) and by the Vector engine via a Schraudolph
    integer approximation (tensor_scalar into int16 bits == bf16 exp),
    splitting the softmax-exp load across two engines.
  - Attention uses the scores-transposed layout: scoresT[k, q] tiles so
    that exp(scoresT) is directly the PV matmul's moving operand, and
    the softmax row sums come for free from a ones-column appended to
    the stationary V tile.
  - Softmax normalization is deferred to a post-attention phase: raw
    PV outputs are staged to SBUF in bf16, then one ACT table switch
    gives hardware Reciprocal for all 16 sum rows, and the normalize
    multiplies + output projection run at the end.
"""

import math

import numpy as np

B, S, D = 2, 2048, 1024
H, DH = 16, 64
NCORE = 8
TP = 4  # head-group shards per batch
HPC = H // TP  # heads per core
DHC = HPC * DH  # 256 = per-core slice of the model dim

_cache = {}

# Schraudolph exp in bf16: int16 bits = A*x + B with A folding the 1/8
# softmax scale; bf16 ULP makes round-vs-trunc immaterial.
SCH_A = 0.125 * (1 << 7) / math.log(2.0)
SCH_B = float(127 * (1 << 7)) - 5.6
# kt tiles handled by the Vector engine (rest go to ACT): 8 of 16.
DVE_KT = frozenset((1, 3, 5, 7, 9, 11, 13, 15))


def _build():
    import concourse.bacc as bacc
    import concourse.mybir as mybir
    import concourse.tile as tile

    F32 = mybir.dt.float32
    F32R = mybir.dt.float32r
    BF16 = mybir.dt.bfloat16
    I16 = mybir.dt.int16
    EXP = mybir.ActivationFunctionType.Exp
    RECIP = mybir.ActivationFunctionType.Reciprocal
    MULT = mybir.AluOpType.mult
    ADD = mybir.AluOpType.add

    nc = bacc.Bacc("TRN2", target_bir_lowering=False, debug=False, num_devices=NCORE)

    # DRAM I/O (all images pre-packed on host)
    xt = nc.dram_tensor("xt", [4, 128, 4096], BF16, kind="ExternalInput").ap()
    wq = nc.dram_tensor("wq", [128, 2048], BF16, kind="ExternalInput").ap()
    wk = nc.dram_tensor("wk", [128, 2048], BF16, kind="ExternalInput").ap()
    wv = nc.dram_tensor("wv", [128, 2048], BF16, kind="ExternalInput").ap()
    wo = nc.dram_tensor("wo", [128, 2048], BF16, kind="ExternalInput").ap()
    bqk = nc.dram_tensor("bqk", [128, 4], F32, kind="ExternalInput").ap()
    bv = nc.dram_tensor("bv", [1, DHC], F32, kind="ExternalInput").ap()
    y = nc.dram_tensor("y", [S, D], BF16, kind="ExternalOutput").ap()

    def act_recip(out_ap, in_ap):
        eng = nc.scalar
        ins = [
            eng.lower_ap(in_ap),
            mybir.ImmediateValue(dtype=F32, value=0.0),
            mybir.ImmediateValue(dtype=F32, value=1.0),
            mybir.ImmediateValue(dtype=F32, value=0.0),
        ]
        outs = [eng.lower_ap(out_ap)]
        return eng.add_instruction(
            mybir.InstActivation(
                name=nc.get_next_instruction_name(),
                func=RECIP,
                ins=ins,
                outs=outs,
            )
        )

    with tile.TileContext(nc) as tc:
        with (
            tc.tile_pool(name="const", bufs=1) as pc,
            tc.tile_pool(name="w", bufs=1) as pw,
            tc.tile_pool(name="x", bufs=4) as px,
            tc.tile_pool(name="qkv", bufs=1) as pqkv,
            tc.tile_pool(name="pt", bufs=5) as ppt,
            tc.tile_pool(name="pti", bufs=5) as ppti,
            tc.tile_pool(name="er", bufs=1) as per,
            tc.tile_pool(name="r", bufs=2) as pr,
            tc.tile_pool(name="out", bufs=3) as pout,
            tc.tile_pool(name="mm", bufs=1, space="PSUM") as pmm,
            tc.tile_pool(name="pse", bufs=1, space="PSUM") as pse,
        ):
            # ---- weights/constants on scalar + gpsimd queues, x on sync ----
            wq_sb = pw.tile([128, 2048], BF16, tag="wq")
            wk_sb = pw.tile([128, 2048], BF16, tag="wk")
            wv_sb = pw.tile([128, 2048], BF16, tag="wv")
            wo_sb = pw.tile([128, 2048], BF16, tag="wo")
            for o in range(0, 2048, 512):
                nc.scalar.dma_start(out=wk_sb[:, o : o + 512], in_=wk[:, o : o + 512])
            bqk_sb = pc.tile([128, 4], F32)
            nc.gpsimd.dma_start(out=bqk_sb[:], in_=bqk[:])
            nc.gpsimd.dma_start(out=wv_sb[:, 0:1024], in_=wv[:, 0:1024])
            nc.gpsimd.dma_start(out=wv_sb[:, 1024:2048], in_=wv[:, 1024:2048])
            bv_sb = pc.tile([128, DHC], F32)
            nc.gpsimd.dma_start(out=bv_sb[:], in_=bv.to_broadcast((128, DHC)))
            nc.gpsimd.dma_start(out=wq_sb[:, 0:1024], in_=wq[:, 0:1024])
            nc.gpsimd.dma_start(out=wq_sb[:, 1024:2048], in_=wq[:, 1024:2048])
            nc.gpsimd.dma_start(out=wo_sb[:], in_=wo[:])

            # ---- persistent activations ----
            qT = pqkv.tile([128, 4096], F32R, tag="qT")
            kT = pqkv.tile([128, 4096], F32R, tag="kT")
            # v (bf16): per seq-tile st: 4 heads x (64 v-cols + ones col)
            vsb = pqkv.tile([128, 16 * (DH + 1) * HPC], BF16, tag="v")
            ones_sb = pc.tile([128, 1], BF16, tag="ones")
            nc.vector.memset(ones_sb[:], 1.0)
            nc.vector.tensor_copy(
                vsb[:].rearrange("p (st h c2) -> p st h c2", st=16, h=HPC)[
                    :, :, :, DH : DH + 1
                ],
                ones_sb[:].to_broadcast((128, 16, HPC, 1)),
            )
            embT = pqkv.tile([128, 4096], BF16, tag="embT")

            # ---- x DMA, chunk 0 split fine for a fast first matmul ----
            # x split into 1MB quarters, alternating the two HWDGE queues so
            # both trigger paths feed DMA engines concurrently
            xc_tiles = []
            for c in range(4):
                xc = px.tile([128, 4096], BF16, tag="xc", name=f"xc{c}")
                for pi, o in enumerate(range(0, 4096, 1024)):
                    eng = nc.sync if (pi % 2 == 0) else nc.scalar
                    eng.dma_start(out=xc[:, o : o + 1024], in_=xt[c][:, o : o + 1024])
                xc_tiles.append(xc)

            psn = [0]

            def mm_tile():
                psn[0] = (psn[0] + 1) % 3
                return pmm.tile(
                    [128, 1024], F32, tag=f"s{psn[0]}", name=f"ps{psn[0]}_{nc.next_id()}"
                )

            def qk_proj(proj, wsb, tsb, boff, g2, scp):
                chunks = (2 * scp, 2 * scp + 1)
                ps = mm_tile()
                for ci, c in enumerate(chunks):
                    for kt in range(8):
                        nc.tensor.matmul(
                            ps[:, ci * 512 : ci * 512 + 512],
                            wsb[:, kt * 256 + g2 * 128 : kt * 256 + g2 * 128 + 128],
                            xc_tiles[c][:, kt * 512 : kt * 512 + 512],
                            start=(kt == 0),
                            stop=(kt == 7),
                        )
                nc.scalar.add(
                    tsb[:, g2 * 2048 + scp * 1024 : g2 * 2048 + scp * 1024 + 1024],
                    ps[:],
                    bqk_sb[:, boff + g2 : boff + g2 + 1],
                )

            def v_proj(scp):
                for c in (2 * scp, 2 * scp + 1):
                    ps = mm_tile()
                    for stl in range(4):
                        for kt in range(8):
                            nc.tensor.matmul(
                                ps[:, stl * 256 : stl * 256 + 256],
                                xc_tiles[c][:, kt * 512 + stl * 128 : kt * 512 + stl * 128 + 128],
                                wv_sb[:, kt * 256 : kt * 256 + 256],
                                start=(kt == 0),
                                stop=(kt == 7),
                            )
                    for stl in range(4):
                        st = c * 4 + stl
                        vo = vsb[:, st * 260 : st * 260 + 260].rearrange(
                            "p (h c2) -> p h c2", h=HPC
                        )[:, :, 0:DH]
                        nc.vector.tensor_tensor(
                            out=vo,
                            in0=ps[:, stl * 256 : stl * 256 + 256].rearrange(
                                "p (h c2) -> p h c2", h=HPC
                            ),
                            in1=bv_sb[:].rearrange("p (h c2) -> p h c2", h=HPC),
                            op=ADD,
                        )

            # ---- pre-phase: K, V, Q projections (K first: attention dep) ----
            for scp in range(2):
                for g2 in range(2):
                    qk_proj("k", wk_sb, kT, 2, g2, scp)
                v_proj(scp)
                for g2 in range(2):
                    qk_proj("q", wq_sb, qT, 0, g2, scp)

            # ---- attention: one flattened (block, kt) stream ----
            # All raw PV outputs stage into one [65, 16K] bf16 tile: block
            # blk = g2*4+j, head hh owns columns (blk*2+hh)*512, so the
            # softmax sums of the whole kernel form the single contiguous
            # row 64 and ONE end-of-attention ACT reciprocal covers all.
            # QK+exp run LOOKAHEAD units ahead of PV in the in-order tensor
            # queue so the two exp engines always overlap, including across
            # block boundaries.
            er_all = per.tile([65, 16384], BF16, tag="er")
            pacc_map = {}
            exp_views = {}

            def emit_qk_exp(g2, j, kt):
                jo = g2 * 2048 + j * 512
                ko = g2 * 2048 + kt * 128
                ps = mm_tile()
                for hh in range(2):
                    plo = hh * 64
                    nc.tensor.matmul(
                        ps[:, hh * 512 : hh * 512 + 512],
                        kT[plo : plo + 64, ko : ko + 128],
                        qT[plo : plo + 64, jo : jo + 512],
                        start=True,
                        stop=True,
                    )
                if kt in DVE_KT:
                    pti = ppti.tile(
                        [128, 1024], I16, tag="pti", name=f"pti_{g2}_{j}_{kt}"
                    )
                    nc.vector.tensor_scalar(pti[:], ps[:], SCH_A, SCH_B, MULT, ADD)
                    exp_views[(g2, j, kt)] = pti[:].bitcast(BF16)
                else:
                    pt = ppt.tile(
                        [128, 1024], BF16, tag="pt", name=f"pt_{g2}_{j}_{kt}"
                    )
                    nc.scalar.activation(pt[:], ps[:], EXP, scale=0.125)
                    exp_views[(g2, j, kt)] = pt[:]

            def emit_pv(g2, j, kt):
                if kt == 0:
                    pacc_map[(g2, j)] = [
                        pse.tile([65, 512], F32, tag=f"pse{hh}", name=f"pse{hh}_{g2}_{j}")
                        for hh in range(2)
                    ]
                pacc = pacc_map[(g2, j)]
                ptm = exp_views.pop((g2, j, kt))
                for hh in range(2):
                    h = 2 * g2 + hh
                    nc.tensor.matmul(
                        pacc[hh][:],
                        vsb[:, kt * 260 + h * 65 : kt * 260 + h * 65 + 65],
                        ptm[:, hh * 512 : hh * 512 + 512],
                        start=(kt == 0),
                        stop=(kt == 15),
                    )
                if kt == 15:
                    blk = g2 * 4 + j
                    for hh in range(2):
                        dst = er_all[
                            :, (blk * 2 + hh) * 512 : (blk * 2 + hh) * 512 + 512
                        ]
                        if hh == 0:
                            nc.vector.tensor_copy(dst, pacc[hh][:])
                        else:
                            nc.scalar.copy(dst, pacc[hh][:])

            LOOKAHEAD = 4
            seq = [
                (g2, j, kt) for g2 in range(2) for j in range(4) for kt in range(16)
            ]
            for gi, unit in enumerate(seq):
                emit_qk_exp(*unit)
                if gi >= LOOKAHEAD:
                    emit_pv(*seq[gi - LOOKAHEAD])
            for unit in seq[-LOOKAHEAD:]:
                emit_pv(*unit)

            # ---- post phase: one reciprocal over all sums, then normalize ----
            # DMA-reshape the 16K sums row to [16, 1024] (one block per
            # partition) so the single ACT reciprocal runs 16 lanes wide.
            s16 = pr.tile([16, 1024], BF16, tag="s16")
            nc.sync.dma_start(
                out=s16[:],
                in_=er_all[64:65, :].rearrange("p (b f) -> p b f", b=16),
            )
            s16r = pr.tile([16, 1024], BF16, tag="s16r")
            act_recip(s16r[:], s16[:])

            def norm(g2, j):
                blk = g2 * 4 + j
                jo = g2 * 2048 + j * 512
                t01 = pr.tile([1, 1024], BF16, tag="t01", name=f"t01_{g2}_{j}")
                nc.sync.dma_start(out=t01[:], in_=s16r[blk : blk + 1, :])
                rb = pr.tile([64, 1024], BF16, tag="rb", name=f"rb_{g2}_{j}")
                nc.gpsimd.partition_broadcast(rb[:], t01[:])
                for hh in range(2):
                    plo = hh * 64
                    nc.vector.tensor_tensor(
                        out=embT[plo : plo + 64, jo : jo + 512],
                        in0=er_all[0:64, (blk * 2 + hh) * 512 : (blk * 2 + hh) * 512 + 512],
                        in1=rb[:, hh * 512 : hh * 512 + 512],
                        op=MULT,
                    )

            def oproj(j):
                for qt in range(4 * j, 4 * j + 4):
                    ps_o = mm_tile()
                    for do in range(2):
                        for g2 in range(2):
                            nc.tensor.matmul(
                                ps_o[:, do * 512 : do * 512 + 512],
                                embT[:, g2 * 2048 + qt * 128 : g2 * 2048 + qt * 128 + 128],
                                wo_sb[:, g2 * 1024 + do * 512 : g2 * 1024 + do * 512 + 512],
                                start=(g2 == 0),
                                stop=(g2 == 1),
                            )
                    ot = pout.tile([128, 1024], BF16, tag="ot", name=f"ot{qt}")
                    nc.vector.tensor_copy(ot[:, 0:512], ps_o[:, 0:512])
                    nc.sync.dma_start(out=y[qt * 128 : qt * 128 + 128, 0:512], in_=ot[:, 0:512])
                    nc.scalar.copy(ot[:, 512:1024], ps_o[:, 512:1024])
                    nc.gpsimd.dma_start(out=y[qt * 128 : qt * 128 + 128, 512:1024], in_=ot[:, 512:1024])

            for j in range(4):
                norm(0, j)
                norm(1, j)
                oproj(j)

    nc.compile()
    return nc


def _pack_inputs(x, Wq, bq, Wk, bk, Wv, bv, Wo, bo):
    """Per-core host-side sharding into the exact DMA images (bf16)."""
    import ml_dtypes

    BF = ml_dtypes.bfloat16

    def img_w(Wslice):  # [1024, 256] -> [128, 8*256]
        return np.ascontiguousarray(
            Wslice.reshape(8, 128, DHC).transpose(1, 0, 2).reshape(128, 8 * DHC)
        ).astype(BF)

    in_maps = []
    for i in range(NCORE):
        b, g = i // TP, i % TP
        sl = slice(g * DHC, (g + 1) * DHC)
        xT = x[b].T  # [1024, 2048]
        xt_img = np.ascontiguousarray(
            xT.reshape(8, 128, 4, 512).transpose(2, 1, 0, 3).reshape(4, 128, 4096)
        ).astype(BF)
        bq_rs = bq[sl].reshape(2, 128).T  # [128, 2]
        bk_rs = bk[sl].reshape(2, 128).T
        bqk_img = np.ascontiguousarray(np.concatenate([bq_rs, bk_rs], axis=1))
        wo_img = np.ascontiguousarray(
            Wo[sl, :].reshape(2, 128, D).transpose(1, 0, 2).reshape(128, 2 * D)
        ).astype(BF)
        in_maps.append(
            {
                "xt": xt_img,
                "wq": img_w(Wq[:, sl]),
                "wk": img_w(Wk[:, sl]),
                "wv": img_w(Wv[:, sl]),
                "wo": wo_img,
                "bqk": bqk_img,
                "bv": np.ascontiguousarray(bv[sl].reshape(1, DHC)),
            }
        )
    return in_maps


def kernel(x, Wq, bq, Wk, bk, Wv, bv, Wo, bo, _trace=False):
    from concourse.bass_utils import run_bass_kernel_spmd

    args = [np.asarray(a, dtype=np.float32) for a in (x, Wq, bq, Wk, bk, Wv, bv, Wo, bo)]
    if "nc" not in _cache:
        _cache["nc"] = _build()
    nc = _cache["nc"]

    in_maps = _pack_inputs(*args)
    res = run_bass_kernel_spmd(nc, in_maps, list(range(NCORE)), trace=_trace)
    _cache["last_result"] = res

    out = np.zeros((B, S, D), dtype=np.float32)
    for i in range(NCORE):
        out[i // TP] += res.results[i]["y"].astype(np.float32)
    out += np.asarray(args[8])  # bo, added once per (b, s) row on the host
    return out



# revision 7
# speedup vs baseline: 1.0436x; 1.0436x over previous
"""Multi-head self-attention on 8 Trainium2 NeuronCores.

Sharding: core i handles batch b = i // 4 and head-group g = i % 4
(4 of 16 heads).  Tensor-parallel over heads for the QKV/attention/output
projection, data-parallel over batch.  Each core produces a partial
output (its head-group's slice of the final projection); the all-reduce
over the 4 head-group partials per batch happens on the host after the
gather, together with adding the output bias exactly once.

Device layout notes:
  - Host pre-transposes x to xT and pre-packs every tensor into the
    exact [128, F] SBUF image the kernel DMAs, so the device never
    transposes anything.
  - QK matmuls are fp32r (full-rate single-pass fp32); the PV side runs
    in bf16: exp(scores) is produced in bf16 both by the ACT engine
    (exact spline exp) and by the Vector engine via a Schraudolph
    integer approximation (tensor_scalar into int16 bits == bf16 exp),
    splitting the softmax-exp load across two engines.
  - Attention uses the scores-transposed layout: scoresT[k, q] tiles so
    that exp(scoresT) is directly the PV matmul's moving operand, and
    the softmax row sums come for free from a ones-column appended to
    the stationary V tile.
  - Softmax normalization is deferred to a post-attention phase: raw
    PV outputs are staged to SBUF in bf16, then one ACT table switch
    gives hardware Reciprocal for all 16 sum rows, and the normalize
    multiplies + output projection run at the end.
"""

import math

import numpy as np

B, S, D = 2, 2048, 1024
H, DH = 16, 64
NCORE = 8
TP = 4  # head-group shards per batch
HPC = H // TP  # heads per core
DHC = HPC * DH  # 256 = per-core slice of the model dim

_cache = {}

# Schraudolph exp in bf16: int16 bits = A*x + B with A folding the 1/8
# softmax scale; bf16 ULP makes round-vs-trunc immaterial.
SCH_A = 0.125 * (1 << 7) / math.log(2.0)
SCH_B = float(127 * (1 << 7)) - 5.6
# kt tiles handled by the Vector engine (rest go to ACT): 7 of 16
# (DVE's Schraudolph tile is ~1.2x slower than ACT's exp tile).
DVE_KT = frozenset((1, 3, 5, 7, 9, 11, 13))


def _build():
    import concourse.bacc as bacc
    import concourse.mybir as mybir
    import concourse.tile as tile

    F32 = mybir.dt.float32
    F32R = mybir.dt.float32r
    BF16 = mybir.dt.bfloat16
    I16 = mybir.dt.int16
    EXP = mybir.ActivationFunctionType.Exp
    RECIP = mybir.ActivationFunctionType.Reciprocal
    MULT = mybir.AluOpType.mult
    ADD = mybir.AluOpType.add

    nc = bacc.Bacc("TRN2", target_bir_lowering=False, debug=False, num_devices=NCORE)

    # DRAM I/O (all images pre-packed on host)
    xt = nc.dram_tensor("xt", [4, 128, 4096], BF16, kind="ExternalInput").ap()
    wq = nc.dram_tensor("wq", [128, 2048], BF16, kind="ExternalInput").ap()
    wk = nc.dram_tensor("wk", [128, 2048], BF16, kind="ExternalInput").ap()
    wv = nc.dram_tensor("wv", [128, 2048], BF16, kind="ExternalInput").ap()
    wo = nc.dram_tensor("wo", [128, 2048], BF16, kind="ExternalInput").ap()
    bqk = nc.dram_tensor("bqk", [128, 4], F32, kind="ExternalInput").ap()
    bv = nc.dram_tensor("bv", [1, DHC], F32, kind="ExternalInput").ap()
    y = nc.dram_tensor("y", [S, D], BF16, kind="ExternalOutput").ap()

    def act_recip(out_ap, in_ap):
        eng = nc.scalar
        ins = [
            eng.lower_ap(in_ap),
            mybir.ImmediateValue(dtype=F32, value=0.0),
            mybir.ImmediateValue(dtype=F32, value=1.0),
            mybir.ImmediateValue(dtype=F32, value=0.0),
        ]
        outs = [eng.lower_ap(out_ap)]
        return eng.add_instruction(
            mybir.InstActivation(
                name=nc.get_next_instruction_name(),
                func=RECIP,
                ins=ins,
                outs=outs,
            )
        )

    with tile.TileContext(nc) as tc:
        with (
            tc.tile_pool(name="const", bufs=1) as pc,
            tc.tile_pool(name="w", bufs=1) as pw,
            tc.tile_pool(name="x", bufs=4) as px,
            tc.tile_pool(name="qkv", bufs=1) as pqkv,
            tc.tile_pool(name="pt", bufs=5) as ppt,
            tc.tile_pool(name="pti", bufs=5) as ppti,
            tc.tile_pool(name="er", bufs=1) as per,
            tc.tile_pool(name="r", bufs=2) as pr,
            tc.tile_pool(name="out", bufs=3) as pout,
            tc.tile_pool(name="mm", bufs=1, space="PSUM") as pmm,
            tc.tile_pool(name="pse", bufs=1, space="PSUM") as pse,
        ):
            # ---- weights/constants DMA; x spread over 4 queues chunk-major ----
            wq_sb = pw.tile([128, 2048], BF16, tag="wq")
            wk_sb = pw.tile([128, 2048], BF16, tag="wk")
            wv_sb = pw.tile([128, 2048], BF16, tag="wv")
            wo_sb = pw.tile([128, 2048], BF16, tag="wo")
            # wk's first quarter lands first (warmup + k-proj need it first)
            nc.scalar.dma_start(out=wk_sb[:, 0:512], in_=wk[:, 0:512])

            # ---- persistent activations ----
            qT = pqkv.tile([128, 4096], BF16, tag="qT")
            kT = pqkv.tile([128, 4096], BF16, tag="kT")
            # v (bf16): per seq-tile st: 4 heads x (64 v-cols + ones col)
            vsb = pqkv.tile([128, 16 * (DH + 1) * HPC], BF16, tag="v")
            ones_sb = pc.tile([128, 1], BF16, tag="ones")
            nc.vector.memset(ones_sb[:], 1.0)
            nc.vector.tensor_copy(
                vsb[:].rearrange("p (st h c2) -> p st h c2", st=16, h=HPC)[
                    :, :, :, DH : DH + 1
                ],
                ones_sb[:].to_broadcast((128, 16, HPC, 1)),
            )
            embT = pqkv.tile([128, 4096], BF16, tag="embT")

            # ---- x DMA: 16 quarter-chunk pieces round-robin over the three
            # DMA-capable queues (sync / scalar / gpsimd), chunk-major so
            # chunk 0 completes first; weight DMAs slotted in behind the
            # pieces so each arrives just before its consumer ----
            xc_tiles = [
                px.tile([128, 4096], BF16, tag="xc", name=f"xc{c}") for c in range(4)
            ]
            bqk_sb = pc.tile([128, 4], F32)
            bv_sb = pc.tile([128, DHC], F32)
            nc.sync.dma_start(out=bqk_sb[:], in_=bqk[:])
            xq = (nc.sync, nc.scalar, nc.gpsimd)
            qi = 1
            for c in range(4):
                for o in range(0, 4096, 1024):
                    xq[qi % 3].dma_start(
                        out=xc_tiles[c][:, o : o + 1024], in_=xt[c][:, o : o + 1024]
                    )
                    qi += 1
                if c == 0:
                    for o in range(512, 2048, 512):
                        nc.scalar.dma_start(
                            out=wk_sb[:, o : o + 512], in_=wk[:, o : o + 512]
                        )
                    nc.gpsimd.dma_start(out=wv_sb[:, 0:1024], in_=wv[:, 0:1024])
                    nc.gpsimd.dma_start(out=wv_sb[:, 1024:2048], in_=wv[:, 1024:2048])
                    nc.sync.dma_start(out=bv_sb[:], in_=bv.to_broadcast((128, DHC)))
                if c == 1:
                    nc.scalar.dma_start(out=wq_sb[:, 0:1024], in_=wq[:, 0:1024])
                    nc.scalar.dma_start(out=wq_sb[:, 1024:2048], in_=wq[:, 1024:2048])
                if c == 2:
                    nc.gpsimd.dma_start(out=wo_sb[:], in_=wo[:])

            psn = [0]

            def mm_tile():
                psn[0] = (psn[0] + 1) % 3
                return pmm.tile(
                    [128, 1024], F32, tag=f"s{psn[0]}", name=f"ps{psn[0]}_{nc.next_id()}"
                )

            # ---- PE warm-up: ~27 throwaway matmuls on wk's first quarter
            # while x streams in.  Keeps the HAM activity window busy so the
            # clock gate opens (1.2 -> 2.4 GHz) before the real projections
            # start, instead of ~15us into them.
            for _ in range(3):
                ps_wu = mm_tile()
                for r in range(9):
                    nc.tensor.matmul(
                        ps_wu[:, (r % 2) * 512 : (r % 2) * 512 + 512],
                        wk_sb[:, 0:128],
                        wk_sb[:, 0:512],
                        start=True,
                        stop=True,
                    )

            def qk_proj(proj, wsb, tsb, boff, g2, scp):
                chunks = (2 * scp, 2 * scp + 1)
                ps = mm_tile()
                for ci, c in enumerate(chunks):
                    for kt in range(8):
                        nc.tensor.matmul(
                            ps[:, ci * 512 : ci * 512 + 512],
                            wsb[:, kt * 256 + g2 * 128 : kt * 256 + g2 * 128 + 128],
                            xc_tiles[c][:, kt * 512 : kt * 512 + 512],
                            start=(kt == 0),
                            stop=(kt == 7),
                        )
                nc.scalar.add(
                    tsb[:, g2 * 2048 + scp * 1024 : g2 * 2048 + scp * 1024 + 1024],
                    ps[:],
                    bqk_sb[:, boff + g2 : boff + g2 + 1],
                )

            def v_proj(scp):
                for c in (2 * scp, 2 * scp + 1):
                    ps = mm_tile()
                    for stl in range(4):
                        for kt in range(8):
                            nc.tensor.matmul(
                                ps[:, stl * 256 : stl * 256 + 256],
                                xc_tiles[c][:, kt * 512 + stl * 128 : kt * 512 + stl * 128 + 128],
                                wv_sb[:, kt * 256 : kt * 256 + 256],
                                start=(kt == 0),
                                stop=(kt == 7),
                            )
                    for stl in range(4):
                        st = c * 4 + stl
                        vo = vsb[:, st * 260 : st * 260 + 260].rearrange(
                            "p (h c2) -> p h c2", h=HPC
                        )[:, :, 0:DH]
                        nc.vector.tensor_tensor(
                            out=vo,
                            in0=ps[:, stl * 256 : stl * 256 + 256].rearrange(
                                "p (h c2) -> p h c2", h=HPC
                            ),
                            in1=bv_sb[:].rearrange("p (h c2) -> p h c2", h=HPC),
                            op=ADD,
                        )

            # ---- pre-phase: K, V, Q projections (K first: attention dep) ----
            for scp in range(2):
                for g2 in range(2):
                    qk_proj("k", wk_sb, kT, 2, g2, scp)
                v_proj(scp)
                for g2 in range(2):
                    qk_proj("q", wq_sb, qT, 0, g2, scp)

            # ---- attention: one flattened (block, kt) stream ----
            # All raw PV outputs stage into one [65, 16K] bf16 tile: block
            # blk = g2*4+j, head hh owns columns (blk*2+hh)*512, so the
            # softmax sums of the whole kernel form the single contiguous
            # row 64 and ONE end-of-attention ACT reciprocal covers all.
            # QK+exp run LOOKAHEAD units ahead of PV in the in-order tensor
            # queue so the two exp engines always overlap, including across
            # block boundaries.
            er_all = per.tile([65, 16384], BF16, tag="er")
            pacc_map = {}
            exp_views = {}

            def emit_qk_exp(g2, j, kt):
                jo = g2 * 2048 + j * 512
                ko = g2 * 2048 + kt * 128
                ps = mm_tile()
                for hh in range(2):
                    plo = hh * 64
                    nc.tensor.matmul(
                        ps[:, hh * 512 : hh * 512 + 512],
                        kT[plo : plo + 64, ko : ko + 128],
                        qT[plo : plo + 64, jo : jo + 512],
                        start=True,
                        stop=True,
                    )
                if kt in DVE_KT:
                    pti = ppti.tile(
                        [128, 1024], I16, tag="pti", name=f"pti_{g2}_{j}_{kt}"
                    )
                    nc.vector.tensor_scalar(pti[:], ps[:], SCH_A, SCH_B, MULT, ADD)
                    exp_views[(g2, j, kt)] = pti[:].bitcast(BF16)
                else:
                    pt = ppt.tile(
                        [128, 1024], BF16, tag="pt", name=f"pt_{g2}_{j}_{kt}"
                    )
                    nc.scalar.activation(pt[:], ps[:], EXP, scale=0.125)
                    exp_views[(g2, j, kt)] = pt[:]

            def emit_pv(g2, j, kt):
                if kt == 0:
                    pacc_map[(g2, j)] = [
                        pse.tile([65, 512], F32, tag=f"pse{hh}", name=f"pse{hh}_{g2}_{j}")
                        for hh in range(2)
                    ]
                pacc = pacc_map[(g2, j)]
                ptm = exp_views.pop((g2, j, kt))
                for hh in range(2):
                    h = 2 * g2 + hh
                    nc.tensor.matmul(
                        pacc[hh][:],
                        vsb[:, kt * 260 + h * 65 : kt * 260 + h * 65 + 65],
                        ptm[:, hh * 512 : hh * 512 + 512],
                        start=(kt == 0),
                        stop=(kt == 15),
                    )
                if kt == 15:
                    blk = g2 * 4 + j
                    for hh in range(2):
                        dst = er_all[
                            :, (blk * 2 + hh) * 512 : (blk * 2 + hh) * 512 + 512
                        ]
                        if hh == 0:
                            nc.vector.tensor_copy(dst, pacc[hh][:])
                        else:
                            nc.scalar.copy(dst, pacc[hh][:])

            LOOKAHEAD = 4
            seq = [
                (g2, j, kt) for g2 in range(2) for j in range(4) for kt in range(16)
            ]
            for gi, unit in enumerate(seq):
                emit_qk_exp(*unit)
                if gi >= LOOKAHEAD:
                    emit_pv(*seq[gi - LOOKAHEAD])
            for unit in seq[-LOOKAHEAD:]:
                emit_pv(*unit)

            # ---- post phase: one reciprocal over all sums, then normalize ----
            # DMA-reshape the 16K sums row to [16, 1024] (one block per
            # partition) so the single ACT reciprocal runs 16 lanes wide.
            s16 = pr.tile([16, 1024], BF16, tag="s16")
            nc.sync.dma_start(
                out=s16[:],
                in_=er_all[64:65, :].rearrange("p (b f) -> p b f", b=16),
            )
            s16r = pr.tile([16, 1024], BF16, tag="s16r")
            act_recip(s16r[:], s16[:])

            def norm(g2, j):
                blk = g2 * 4 + j
                jo = g2 * 2048 + j * 512
                t01 = pr.tile([1, 1024], BF16, tag="t01", name=f"t01_{g2}_{j}")
                nc.sync.dma_start(out=t01[:], in_=s16r[blk : blk + 1, :])
                rb = pr.tile([64, 1024], BF16, tag="rb", name=f"rb_{g2}_{j}")
                nc.gpsimd.partition_broadcast(rb[:], t01[:])
                for hh in range(2):
                    plo = hh * 64
                    nc.vector.tensor_tensor(
                        out=embT[plo : plo + 64, jo : jo + 512],
                        in0=er_all[0:64, (blk * 2 + hh) * 512 : (blk * 2 + hh) * 512 + 512],
                        in1=rb[:, hh * 512 : hh * 512 + 512],
                        op=MULT,
                    )

            def oproj(j):
                for qt in range(4 * j, 4 * j + 4):
                    ps_o = mm_tile()
                    for do in range(2):
                        for g2 in range(2):
                            nc.tensor.matmul(
                                ps_o[:, do * 512 : do * 512 + 512],
                                embT[:, g2 * 2048 + qt * 128 : g2 * 2048 + qt * 128 + 128],
                                wo_sb[:, g2 * 1024 + do * 512 : g2 * 1024 + do * 512 + 512],
                                start=(g2 == 0),
                                stop=(g2 == 1),
                            )
                    ot = pout.tile([128, 1024], BF16, tag="ot", name=f"ot{qt}")
                    nc.vector.tensor_copy(ot[:, 0:512], ps_o[:, 0:512])
                    nc.sync.dma_start(out=y[qt * 128 : qt * 128 + 128, 0:512], in_=ot[:, 0:512])
                    nc.scalar.copy(ot[:, 512:1024], ps_o[:, 512:1024])
                    nc.gpsimd.dma_start(out=y[qt * 128 : qt * 128 + 128, 512:1024], in_=ot[:, 512:1024])

            for j in range(4):
                norm(0, j)
                norm(1, j)
                oproj(j)

    nc.compile()
    return nc


def _pack_inputs(x, Wq, bq, Wk, bk, Wv, bv, Wo, bo):
    """Per-core host-side sharding into the exact DMA images (bf16)."""
    import ml_dtypes

    BF = ml_dtypes.bfloat16

    def img_w(Wslice):  # [1024, 256] -> [128, 8*256]
        return np.ascontiguousarray(
            Wslice.reshape(8, 128, DHC).transpose(1, 0, 2).reshape(128, 8 * DHC)
        ).astype(BF)

    in_maps = []
    for i in range(NCORE):
        b, g = i // TP, i % TP
        sl = slice(g * DHC, (g + 1) * DHC)
        xT = x[b].T  # [1024, 2048]
        xt_img = np.ascontiguousarray(
            xT.reshape(8, 128, 4, 512).transpose(2, 1, 0, 3).reshape(4, 128, 4096)
        ).astype(BF)
        bq_rs = bq[sl].reshape(2, 128).T  # [128, 2]
        bk_rs = bk[sl].reshape(2, 128).T
        bqk_img = np.ascontiguousarray(np.concatenate([bq_rs, bk_rs], axis=1))
        wo_img = np.ascontiguousarray(
            Wo[sl, :].reshape(2, 128, D).transpose(1, 0, 2).reshape(128, 2 * D)
        ).astype(BF)
        in_maps.append(
            {
                "xt": xt_img,
                "wq": img_w(Wq[:, sl]),
                "wk": img_w(Wk[:, sl]),
                "wv": img_w(Wv[:, sl]),
                "wo": wo_img,
                "bqk": bqk_img,
                "bv": np.ascontiguousarray(bv[sl].reshape(1, DHC)),
            }
        )
    return in_maps


def kernel(x, Wq, bq, Wk, bk, Wv, bv, Wo, bo, _trace=False):
    from concourse.bass_utils import run_bass_kernel_spmd

    args = [np.asarray(a, dtype=np.float32) for a in (x, Wq, bq, Wk, bk, Wv, bv, Wo, bo)]
    if "nc" not in _cache:
        _cache["nc"] = _build()
    nc = _cache["nc"]

    in_maps = _pack_inputs(*args)
    res = run_bass_kernel_spmd(nc, in_maps, list(range(NCORE)), trace=_trace)
    _cache["last_result"] = res

    out = np.zeros((B, S, D), dtype=np.float32)
    for i in range(NCORE):
        out[i // TP] += res.results[i]["y"].astype(np.float32)
    out += np.asarray(args[8])  # bo, added once per (b, s) row on the host
    return out

